# revision 23
# baseline (speedup 1.0000x reference)
"""TRN2 Bass kernel for nn_Attention_59270548685139 (v5, all-bf16 datapath).

Custom two-stage-normalized attention, B=8, N=1024, D=1024, H=8, DH=64.
Sharding: data-parallel over batch -- one batch element per NeuronCore (8 cores).

Math per batch element (matching the reference):
  q = x @ Wq, k = x @ Wk, v = x @ Wv          (split into 8 heads of 64)
  sim[i,j]  = (q_i . k_j) * DH**-0.5
  attn      = softmax over the QUERY dim i    -> E[i,j]/C[j], C[j] = sum_i E[i,j]
  attn      = attn / (sum_j attn + eps)       -> per-i scale 1/(R[i]+eps)
  out       = attn @ v ; y = out @ Wo + bo

Key design points (v5):
- x and all weights are converted to bf16 ON THE HOST and DMA'd as bf16:
  halves input HBM traffic (12.6 -> 6 MB), gives 1-2KB contiguous DMA runs,
  lets the full Wq/Wk/Wv/Wo live in SBUF (no weight-tile ring), enables FWL
  weight loads, and keeps the PE's HAM clock gate at K=8/8 (fp32r matmuls
  were observed never to un-throttle the 1.2GHz cold clock; bf16 ones do).
  Accumulation stays fp32 in PSUM; the softmax/normalization chain
  (C, 1/C, R, 1/R) stays fp32. Tolerance is 2e-2; measured error ~5e-3.
- Heads are processed in PAIRS living on partitions 0:64 / 64:128 of qt/kt,
  so the two S^T matmuls run CONCURRENTLY on disjoint PE row-groups
  (tile_position (0,0) / (64,0) auto-derived), halving S-phase PE time.
- Scores are computed transposed (S^T[j,i]) so the softmax-over-queries
  reduction fuses into the ACT exp pass (accum_out -> C[j]); ACT is the
  attention-phase bottleneck engine (~90us of exp+accum), so the pair loop
  is emitted jb-granular with V/QK projections interleaved as PE filler.
- The key-dim renormalization folds into V'=V/C (GPSIMD normalize_recip via
  an fp32 scratch -> DVE bf16 cast; the Q7 direct-bf16 write path is broken)
  with an appended 1/C column so attn@v also produces R[i] for free.
- 1/R uses reciprocal_approx_fast (custom DVE op, ~5x faster than
  reciprocal); its input must be staged at partition 0 (the custom-uop path
  ignores nonzero base partitions).
- The last pair's recip/broadcast/mul tail is emitted inline and the output
  projection begins with bias + head-pair-0..2 partial accumulations so the
  PE never idles into the HAM MID window at the attention->output boundary.
"""

import numpy as np

import concourse.bass as bass
import concourse.tile as tile
from concourse import bacc, mybir
from concourse.bass_utils import run_bass_kernel_spmd
from concourse.masks import make_identity

FP32 = mybir.dt.float32
BF16 = mybir.dt.bfloat16

B, N, D = 8, 1024, 1024
H, DH = 8, 64
INNER = H * DH  # 512
SCALE = DH ** -0.5
EPS = 1e-7
P = 128
NCORES = 8

DC = D // P       # 8 contraction chunks over D
IC = INNER // P   # 4 chunks over INNER
NB = N // P       # 8 seq blocks of 128

_NC_CACHE = None


def _build_nc(dbg=False):
    nc = bacc.Bacc("TRN2", target_bir_lowering=False, debug=False)

    x_d = nc.dram_tensor("x", [N, D], BF16, kind="ExternalInput")
    wq_d = nc.dram_tensor("Wq", [D, INNER], BF16, kind="ExternalInput")
    wk_d = nc.dram_tensor("Wk", [D, INNER], BF16, kind="ExternalInput")
    wv_d = nc.dram_tensor("Wv", [D, INNER], BF16, kind="ExternalInput")
    wo_d = nc.dram_tensor("Wo", [INNER, D], BF16, kind="ExternalInput")
    bo_d = nc.dram_tensor("bo", [D], BF16, kind="ExternalInput")
    y_d = nc.dram_tensor("y", [N, D], FP32, kind="ExternalOutput")

    dbg_d = {}
    if dbg:
        for nm, shp, dt in [
            ("dbg_qt0", [P, N], BF16), ("dbg_kt0", [P, N], BF16),
            ("dbg_v0", [P, INNER], FP32), ("dbg_ce", [P, NB], FP32),
            ("dbg_co", [P, NB], FP32), ("dbg_et00", [P, N], BF16),
            ("dbg_v2e", [P, NB, DH + 2], BF16),
            ("dbg_us0", [DH + 1, N], FP32), ("dbg_ot0", [P, N], BF16),
        ]:
            dbg_d[nm] = nc.dram_tensor(nm, shp, dt, kind="ExternalOutput")

    with tile.TileContext(nc) as tc:
        # ---------------- pools ----------------
        const_pool = tc.alloc_tile_pool(name="const", bufs=1)
        qt_pool = tc.alloc_tile_pool(name="qt", bufs=1)
        kt_pool = tc.alloc_tile_pool(name="kt", bufs=1)
        v_pool = tc.alloc_tile_pool(name="v", bufs=1)
        ot_pool = tc.alloc_tile_pool(name="ot", bufs=1)
        xt_pool = tc.alloc_tile_pool(name="xt", bufs=1)
        w_pool = tc.alloc_tile_pool(name="w", bufs=1)
        et_pool = tc.alloc_tile_pool(name="et", bufs=1)
        sm_pool = tc.alloc_tile_pool(name="sm", bufs=2)
        smb_pool = tc.alloc_tile_pool(name="smb", bufs=2)
        usb_pool = tc.alloc_tile_pool(name="usb", bufs=4)
        y_pool = tc.alloc_tile_pool(name="yp", bufs=2)
        ps_pool = tc.alloc_tile_pool(name="ps", bufs=2, space="PSUM")

        # ---------------- constants ----------------
        ident = const_pool.tile([P, P], FP32, tag="ident")
        make_identity(nc, ident[:])
        identb = const_pool.tile([P, P], BF16, tag="identb")
        nc.vector.tensor_copy(identb[:], ident[:])
        ones_f = const_pool.tile([1, P], FP32, tag="ones_f")
        nc.vector.memset(ones_f[:], 1.0)
        ones_b = const_pool.tile([1, P], BF16, tag="ones_b")
        nc.vector.tensor_copy(ones_b[:], ones_f[:])
        # bo as [1, 2, 512] bf16 (free-dim block db = bo[db*512:(db+1)*512])
        bo_b = const_pool.tile([1, 2, 512], BF16, tag="bo_b")
        nc.sync.dma_start(
            out=bo_b[:],
            in_=bo_d.ap().rearrange("(a n) -> a n", a=2)[None, :, :],
        )
        # preload the exp ACT table set while the x DMA streams
        warm = const_pool.tile([1, 2], FP32, tag="warm")
        nc.scalar.activation(warm[:], ones_f[:, 0:2], mybir.ActivationFunctionType.Exp)
        # warm up the GPSIMD ext-isa library (normalize_recip/partition_
        # broadcast pay a ~6us IRAM load on first call otherwise)
        gw = const_pool.tile([P, 3], FP32, tag="gw")
        nc.vector.memset(gw[:], 1.0)
        nc.gpsimd.normalize_recip(gw[:, 2:3], gw[:, 0:1], gw[:, 1:2])
        gwb = const_pool.tile([2, 4], FP32, tag="gwb")
        nc.gpsimd.partition_broadcast(gwb[:], ones_f[:, 0:4])
        kw_sb = const_pool.tile([P, P], BF16, tag="kw_sb")

        # ---------------- persistent intermediates ----------------
        qt = [qt_pool.tile([P, N], BF16, tag=f"qt{m}", name=f"qt{m}") for m in range(IC)]
        kt = [kt_pool.tile([P, N], BF16, tag=f"kt{m}", name=f"kt{m}") for m in range(IC)]
        vts = [v_pool.tile([P, INNER], FP32, tag=f"v{j}", name=f"v{j}") for j in range(NB)]
        ot = [ot_pool.tile([P, N], BF16, tag=f"ot{m}", name=f"ot{m}") for m in range(IC)]
        xt = [xt_pool.tile([P, N], BF16, tag=f"xt{c}", name=f"xt{c}") for c in range(DC)]
        wq_t = w_pool.tile([P, DC, INNER], BF16, tag="wq")
        wk_t = w_pool.tile([P, DC, INNER], BF16, tag="wk")
        wv_t = w_pool.tile([P, DC, INNER], BF16, tag="wv")
        wo_t = w_pool.tile([P, IC, D], BF16, tag="wo")

        # ---------------- phase A: x first, then weights -------------------
        xhs = []
        for ib in range(NB):
            xh = et_pool.tile([P, N], BF16, tag=f"ete{ib}", name=f"xn{ib}")
            nc.sync.dma_start(out=xh[:], in_=x_d.ap()[ib * P:(ib + 1) * P, :])
            xhs.append(xh)
        for w_t, wd in ((wq_t, wq_d), (wk_t, wk_d), (wv_t, wv_d)):
            nc.sync.dma_start(
                out=w_t[:],
                in_=wd.ap().rearrange("(c p) n -> p c n", p=P),
            )
        nc.sync.dma_start(
            out=wo_t[:],
            in_=wo_d.ap().rearrange("(c p) n -> p c n", p=P),
        )

        # dense ident matmuls during the DMA wait: un-throttles the HAM clock
        # gate before the transposes/projections, which then run at 2.4GHz
        p_kw = ps_pool.tile([P, N], FP32, tag="proj", name="pkw", bufs=1)
        for i in range(40):
            nc.tensor.matmul(
                p_kw[:, 0:P], identb[:], identb[:],
                start=True, stop=True, skip_group_check=True,
            )
        nc.vector.tensor_copy(kw_sb[:], p_kw[:, 0:P])

        for ib in range(NB):
            p_t = ps_pool.tile([P, N], BF16, tag="s", name=f"ptp{ib}", bufs=2)
            for c in range(DC):
                nc.tensor.transpose(
                    p_t[:, c * P:(c + 1) * P],
                    xhs[ib][:, c * P:(c + 1) * P],
                    identb[:],
                )
            for c in range(DC):
                src = p_t[:, c * P:(c + 1) * P]
                dst = xt[c][:, ib * P:(ib + 1) * P]
                if c % 2 == 0:
                    nc.scalar.copy(dst, src)
                else:
                    nc.vector.tensor_copy(dst, src)

        # ---------------- Q/K quarter projection helper --------------------
        def emit_qk_half(key, mb, ih, eng="vector"):
            """One ih-half (512 tokens) of a Q/K quarter projection."""
            dst, w_t = (qt, wq_t) if key == "q" else (kt, wk_t)
            p_h = ps_pool.tile([P, N], FP32, tag="proj", name=f"pp{key}{mb}_{ih}", bufs=1)
            for c in range(DC):
                nc.tensor.matmul(
                    p_h[:, 0:512],
                    w_t[:, c, mb * P:(mb + 1) * P],
                    xt[c][:, ih * 512:(ih + 1) * 512],
                    start=(c == 0), stop=(c == DC - 1),
                    skip_group_check=True,
                )
            dap = dst[mb][:, ih * 512:(ih + 1) * 512]
            if eng == "scalar":
                nc.scalar.copy(dap, p_h[:, 0:512])
            else:
                nc.vector.tensor_copy(dap, p_h[:, 0:512])

        # pair 0 projections up front (gate the first S matmuls); drains on
        # the scalar engine -- ACT is idle at startup, DVE is on the S path
        emit_qk_half("q", 0, 0, eng="scalar")
        emit_qk_half("q", 0, 1, eng="scalar")
        emit_qk_half("k", 0, 0, eng="scalar")
        emit_qk_half("k", 0, 1, eng="scalar")

        # ---------------- V projection (256-col groups, 2 seq blocks) -------
        def _v_half(p_v, s, jp, half):
            jb = 2 * jp + half
            for c in range(DC):
                nc.tensor.matmul(
                    p_v[:, half * 256:(half + 1) * 256],
                    xt[c][:, jb * P:(jb + 1) * P],
                    wv_t[:, c, s * 256:(s + 1) * 256],
                    start=(c == 0), stop=(c == DC - 1),
                    skip_group_check=True,
                )

        def _v_drain(p_v, s, jp):
            for half in range(2):
                jb = 2 * jp + half
                nc.vector.tensor_copy(
                    vts[jb][:, s * 256:(s + 1) * 256],
                    p_v[:, half * 256:(half + 1) * 256],
                )

        def emit_v_group(s, jp):
            p_v = ps_pool.tile([P, N], FP32, tag="proj", name=f"pv{s}_{jp}", bufs=1)
            _v_half(p_v, s, jp, 0)
            _v_half(p_v, s, jp, 1)
            _v_drain(p_v, s, jp)

        def make_v_chunks(s, jp):
            cell = {}

            def c1():
                p_v = ps_pool.tile([P, N], FP32, tag="proj",
                                   name=f"pv{s}_{jp}", bufs=1)
                cell["p"] = p_v
                _v_half(p_v, s, jp, 0)

            def c2():
                _v_half(cell["p"], s, jp, 1)
                _v_drain(cell["p"], s, jp)

            return [c1, c2]

        # ---------------- attention: head pairs ----------------------------
        us_tiles = {}
        rrec_tiles = {}

        def emit_recip(h):
            # the custom-DVE recip needs its input at partition 0 (it does not
            # honor a nonzero base partition), so stage the R row there first
            rin = smb_pool.tile([1, N], FP32, tag="rin", name=f"ri{h}", bufs=1)
            nc.vector.tensor_copy(rin[:], us_tiles[h][DH:DH + 1, :])
            rrec = smb_pool.tile([1, N], FP32, tag="rrec", name=f"rr{h}", bufs=1)
            nc.vector.reciprocal_approx_fast(rrec[:], rin[:])
            rrec_tiles[h] = rrec

        def emit_finish(h):
            gmb, goff = h // 2, (h % 2) * DH
            bc_sb = smb_pool.tile([DH, N], FP32, tag="bc", name=f"bcs{h}", bufs=1)
            nc.gpsimd.partition_broadcast(bc_sb[:], rrec_tiles[h][:])
            nc.vector.tensor_mul(
                ot[gmb][goff:goff + DH, :],
                us_tiles[h][0:DH, :],
                bc_sb[:],
            )

        def make_qk_chunks(key, mb, ih):
            cell = {}
            dst, w_t = (qt, wq_t) if key == "q" else (kt, wk_t)

            def c1():
                p_h = ps_pool.tile([P, N], FP32, tag="proj",
                                   name=f"pp{key}{mb}_{ih}", bufs=1)
                cell["p"] = p_h
                for c in range(4):
                    nc.tensor.matmul(
                        p_h[:, 0:512],
                        w_t[:, c, mb * P:(mb + 1) * P],
                        xt[c][:, ih * 512:(ih + 1) * 512],
                        start=(c == 0), stop=False, skip_group_check=True,
                    )

            def c2():
                p_h = cell["p"]
                for c in range(4, DC):
                    nc.tensor.matmul(
                        p_h[:, 0:512],
                        w_t[:, c, mb * P:(mb + 1) * P],
                        xt[c][:, ih * 512:(ih + 1) * 512],
                        start=False, stop=(c == DC - 1), skip_group_check=True,
                    )
                nc.vector.tensor_copy(
                    dst[mb][:, ih * 512:(ih + 1) * 512], p_h[:, 0:512]
                )

            return [c1, c2]

        def emit_keepwarm(n):
            p_k = ps_pool.tile([P, N], FP32, tag="proj", name=f"pkw{n}", bufs=1)
            for i in range(8):
                nc.tensor.matmul(
                    p_k[:, 0:P], identb[:], identb[:],
                    start=True, stop=True, skip_group_check=True,
                )
            nc.vector.tensor_copy(kw_sb[:], p_k[:, 0:P])

        # filler units per pair (each ~1-2us of PE work, one per jb slot, so
        # the PE never starves while ACT chews the exps). V pass A is emitted
        # inline at even-jb tops of pair 0 (producers must precede consumers
        # in emission order for the dep tracker).
        def qk_quarter_chunks(mb):
            return (
                make_qk_chunks("q", mb, 0) + make_qk_chunks("q", mb, 1)
                + make_qk_chunks("k", mb, 0) + make_qk_chunks("k", mb, 1)
            )

        def interleave(a, b):
            out = []
            for i in range(max(len(a), len(b))):
                if i < len(a):
                    out.append(a[i])
                if i < len(b):
                    out.append(b[i])
            return out

        fillers = {
            0: qk_quarter_chunks(1),
            1: interleave(
                qk_quarter_chunks(2),
                make_v_chunks(1, 0) + make_v_chunks(1, 1)
                + make_v_chunks(1, 2) + make_v_chunks(1, 3),
            ),
            2: qk_quarter_chunks(3),
            3: [lambda n=n: emit_keepwarm(n) for n in range(8)],
        }

        hoisted_s = {}
        for pr in range(4):
            e, o = 2 * pr, 2 * pr + 1
            if dbg and pr == 1:
                nc.sync.dma_start(out=dbg_d["dbg_us0"].ap(), in_=us_tiles[0][:])
            if pr >= 1:
                emit_recip(e - 2)
                emit_finish(e - 2)
                emit_recip(o - 2)
                emit_finish(o - 2)

            kth_e = kt[pr][0:DH, :]
            qth_e = qt[pr][0:DH, :]
            kth_o = kt[pr][DH:P, :]
            qth_o = qt[pr][DH:P, :]

            c_e = sm_pool.tile([P, NB], FP32, tag="c_e", name=f"ce{pr}", bufs=2)
            c_o = sm_pool.tile([P, NB], FP32, tag="c_o", name=f"co{pr}", bufs=2)
            v2_e = sm_pool.tile([P, NB, DH + 2], BF16, tag="v2_e", name=f"v2e{pr}", bufs=2)
            v2_o = sm_pool.tile([P, NB, DH + 2], BF16, tag="v2_o", name=f"v2o{pr}", bufs=2)
            pu_e0 = ps_pool.tile([DH + 1, 512], FP32, tag="u", name=f"ue0_{pr}", bufs=2)
            pu_e1 = ps_pool.tile([DH + 1, 512], FP32, tag="u", name=f"ue1_{pr}", bufs=2)
            ets_e, ets_o = [], []
            units = fillers[pr]
            ui = 0

            def emit_s_quad(spr, sjb):
                jsl = slice(sjb * P, (sjb + 1) * P)
                se = ps_pool.tile([P, N], FP32, tag="s", name=f"se{spr}_{sjb}", bufs=2)
                so = ps_pool.tile([P, N], FP32, tag="s", name=f"so{spr}_{sjb}", bufs=2)
                # concurrent row-group pair: head e rows 0-63, head o 64-127
                for ih in range(2):
                    nc.tensor.matmul(
                        se[:, ih * 512:(ih + 1) * 512],
                        kt[spr][0:DH, jsl], qt[spr][0:DH, ih * 512:(ih + 1) * 512],
                        start=True, stop=True, skip_group_check=True,
                    )
                for ih in range(2):
                    nc.tensor.matmul(
                        so[:, ih * 512:(ih + 1) * 512],
                        kt[spr][DH:P, jsl], qt[spr][DH:P, ih * 512:(ih + 1) * 512],
                        start=True, stop=True, skip_group_check=True,
                    )
                return se, so

            for jb in range(NB):
                jsl = slice(jb * P, (jb + 1) * P)
                if (pr, jb) in hoisted_s:
                    p_se, p_so = hoisted_s.pop((pr, jb))
                else:
                    p_se, p_so = emit_s_quad(pr, jb)
                # V pass A inline at even-jb tops of pair 0 (producers must
                # precede their consumer normalizes in emission order)
                if pr == 0 and jb % 2 == 0:
                    emit_v_group(0, jb // 2)
                et_e = et_pool.tile([P, N], BF16, tag=f"ete{jb}", name=f"ete{pr}_{jb}")
                nc.scalar.activation(
                    et_e[:], p_se[:], mybir.ActivationFunctionType.Exp,
                    scale=SCALE, accum_out=c_e[:, jb:jb + 1],
                )
                et_o = et_pool.tile([P, N], BF16, tag=f"eto{jb}", name=f"eto{pr}_{jb}")
                nc.scalar.activation(
                    et_o[:], p_so[:], mybir.ActivationFunctionType.Exp,
                    scale=SCALE, accum_out=c_o[:, jb:jb + 1],
                )
                ets_e.append(et_e)
                ets_o.append(et_o)
                # V' = V / C[j] on GPSIMD (fp32 scratch; the Q7 bf16 write
                # path is broken) -> DVE cast to bf16; c becomes 1/C in place
                nv_e = sm_pool.tile([P, DH], FP32, tag="nv_e", name=f"nve{pr}_{jb}", bufs=2)
                nc.gpsimd.normalize_recip(
                    nv_e[:], vts[jb][:, e * DH:(e + 1) * DH], c_e[:, jb:jb + 1]
                )
                nc.vector.tensor_copy(v2_e[:, jb, 0:DH], nv_e[:])
                nc.vector.tensor_copy(v2_e[:, jb, DH:DH + 1], c_e[:, jb:jb + 1])
                nv_o = sm_pool.tile([P, DH], FP32, tag="nv_o", name=f"nvo{pr}_{jb}", bufs=2)
                nc.gpsimd.normalize_recip(
                    nv_o[:], vts[jb][:, o * DH:(o + 1) * DH], c_o[:, jb:jb + 1]
                )
                nc.vector.tensor_copy(v2_o[:, jb, 0:DH], nv_o[:])
                nc.vector.tensor_copy(v2_o[:, jb, DH:DH + 1], c_o[:, jb:jb + 1])
                # U^T accumulation for head e (both token halves); row DH = R
                nc.tensor.matmul(
                    pu_e0[:], v2_e[:, jb, 0:DH + 1], et_e[:, 0:512],
                    start=(jb == 0), stop=(jb == NB - 1), skip_group_check=True,
                )
                nc.tensor.matmul(
                    pu_e1[:], v2_e[:, jb, 0:DH + 1], et_e[:, 512:1024],
                    start=(jb == 0), stop=(jb == NB - 1), skip_group_check=True,
                )
                take = 2 if len(units) > NB else 1
                for _ in range(take):
                    if ui < len(units):
                        units[ui]()
                        ui += 1
            while ui < len(units):
                units[ui]()
                ui += 1

            # hoist the next pair's first S-quads ahead of the U_o blocks in
            # the PE FIFO so its exps start as soon as this pair's s-slots
            # free (ACT stays continuous across the pair boundary)
            if pr < 3:
                hoisted_s[(pr + 1, 0)] = emit_s_quad(pr + 1, 0)
                hoisted_s[(pr + 1, 1)] = emit_s_quad(pr + 1, 1)

            # drain head e; its U finished inside the jb loop
            us_e = usb_pool.tile([DH + 1, N], FP32, tag="usb", name=f"us{e}", bufs=4)
            nc.vector.tensor_copy(us_e[:, 0:512], pu_e0[:])
            nc.vector.tensor_copy(us_e[:, 512:1024], pu_e1[:])
            us_tiles[e] = us_e
            if pr == 3:
                emit_recip(e)
                emit_finish(e)

            us_o = usb_pool.tile([DH + 1, N], FP32, tag="usb", name=f"us{o}", bufs=4)
            pu_o0 = ps_pool.tile([DH + 1, 512], FP32, tag="u", name=f"uo0_{pr}", bufs=2)
            for jb in range(NB):
                nc.tensor.matmul(
                    pu_o0[:], v2_o[:, jb, 0:DH + 1], ets_o[jb][:, 0:512],
                    start=(jb == 0), stop=(jb == NB - 1), skip_group_check=True,
                )
            pu_o1 = ps_pool.tile([DH + 1, 512], FP32, tag="u", name=f"uo1_{pr}", bufs=2)
            for jb in range(NB):
                nc.tensor.matmul(
                    pu_o1[:], v2_o[:, jb, 0:DH + 1], ets_o[jb][:, 512:1024],
                    start=(jb == 0), stop=(jb == NB - 1), skip_group_check=True,
                )
            nc.vector.tensor_copy(us_o[:, 0:512], pu_o0[:])
            nc.vector.tensor_copy(us_o[:, 512:1024], pu_o1[:])
            us_tiles[o] = us_o
            if pr == 3:
                emit_recip(o)
                for n in range(4):
                    emit_keepwarm(100 + n)
                emit_finish(o)
            if dbg and pr == 0:
                nc.sync.dma_start(out=dbg_d["dbg_ce"].ap(), in_=c_e[:])
                nc.sync.dma_start(out=dbg_d["dbg_co"].ap(), in_=c_o[:])
                nc.sync.dma_start(out=dbg_d["dbg_et00"].ap(), in_=ets_e[0][:])
                nc.sync.dma_start(out=dbg_d["dbg_v2e"].ap(), in_=v2_e[:])

        # ---------------- output projection (+bo via K=1 matmul) ----------
        def emit_py_partial(ib, mbis, start, stop):
            p_y = py_tiles[ib]
            for db in range(2):
                dsl = slice(db * 512, (db + 1) * 512)
                if start:
                    nc.tensor.matmul(
                        p_y[:, dsl], ones_b[:], bo_b[:, db, :],
                        start=True, stop=False, skip_group_check=True,
                    )
                for mbi in mbis:
                    nc.tensor.matmul(
                        p_y[:, dsl],
                        ot[mbi][:, ib * P:(ib + 1) * P],
                        wo_t[:, mbi, dsl],
                        start=False, stop=(stop and mbi == mbis[-1]),
                        skip_group_check=True,
                    )

        def emit_py_drain(ib):
            p_y = py_tiles[ib]
            for db in range(2):
                dsl = slice(db * 512, (db + 1) * 512)
                y_t = y_pool.tile([P, 512], FP32, tag="y", name=f"y{ib}_{db}")
                nc.scalar.copy(y_t[:], p_y[:, dsl])
                nc.sync.dma_start(
                    out=y_d.ap()[ib * P:(ib + 1) * P, dsl],
                    in_=y_t[:],
                )

        py_tiles = {}
        # first two tiles: bias + pairs 0-2 run while the last pair's tail
        # (us copies -> recip -> broadcast -> mul) drains; pair 3 joins last.
        for ib in (0, 1):
            py_tiles[ib] = ps_pool.tile([P, N], FP32, tag="s", name=f"py{ib}", bufs=2)
            emit_py_partial(ib, [0, 1, 2], start=True, stop=False)
        for ib in (0, 1):
            emit_py_partial(ib, [3], start=False, stop=True)
            emit_py_drain(ib)
        for ib in range(2, NB):
            py_tiles[ib] = ps_pool.tile([P, N], FP32, tag="s", name=f"py{ib}", bufs=2)
            emit_py_partial(ib, [0, 1, 2, 3], start=True, stop=True)
            emit_py_drain(ib)

        if dbg:
            nc.sync.dma_start(out=dbg_d["dbg_qt0"].ap(), in_=qt[0][:])
            nc.sync.dma_start(out=dbg_d["dbg_kt0"].ap(), in_=kt[0][:])
            nc.sync.dma_start(out=dbg_d["dbg_v0"].ap(), in_=vts[0][:])
            nc.sync.dma_start(out=dbg_d["dbg_ot0"].ap(), in_=ot[0][:])

        for p in (ps_pool, y_pool, usb_pool, smb_pool, sm_pool, et_pool,
                  w_pool, xt_pool, ot_pool, v_pool,
                  kt_pool, qt_pool, const_pool):
            p.release()

    nc.finalize()
    return nc


def _get_nc():
    global _NC_CACHE
    if _NC_CACHE is None:
        _NC_CACHE = _build_nc()
    return _NC_CACHE


def _bf16(a):
    import ml_dtypes
    return np.asarray(a, dtype=np.float32).astype(ml_dtypes.bfloat16)


def kernel(x, Wq, Wk, Wv, Wo, bo, _trace=False, **trace_kwargs):
    x = _bf16(x)
    Wq = _bf16(Wq)
    Wk = _bf16(Wk)
    Wv = _bf16(Wv)
    Wo = _bf16(Wo)
    bo = _bf16(bo)

    nc = _get_nc()
    in_maps = [
        {"x": x[c], "Wq": Wq, "Wk": Wk, "Wv": Wv, "Wo": Wo, "bo": bo}
        for c in range(NCORES)
    ]
    res = run_bass_kernel_spmd(
        nc, in_maps, core_ids=list(range(NCORES)), trace=_trace, **trace_kwargs
    )
    out = np.stack([res.results[c]["y"] for c in range(NCORES)], axis=0)
    if _trace:
        return out.astype(np.float32), res
    return out.astype(np.float32)


if __name__ == "__main__":
    rng = np.random.default_rng(0)
    xs = rng.standard_normal((B, N, D), dtype=np.float32)
    wq = rng.standard_normal((D, INNER), dtype=np.float32) * D ** -0.5
    wk = rng.standard_normal((D, INNER), dtype=np.float32) * D ** -0.5
    wv = rng.standard_normal((D, INNER), dtype=np.float32) * D ** -0.5
    wo = rng.standard_normal((INNER, D), dtype=np.float32) * INNER ** -0.5
    bz = np.zeros((D,), dtype=np.float32)
    y = kernel(xs, wq, wk, wv, wo, bz)
    print("ran ok", y.shape, float(np.abs(y).mean()))
